# revision 3
# baseline (speedup 1.0000x reference)
"""HeteroLinear (per-token expert linear) on 8 TRN2 NeuronCores.

Strategy: expert-parallel. The reference computes all 8 GEMMs on every
token and masks (8x redundant compute). Here the host routes tokens to
their expert: tokens of type t go to core t, padded to a static
capacity C. Each core then runs ONE dense [C,1024]@[1024,1024] GEMM in
bf16 — the algorithmic minimum of compute — with the bias add fused
into the PSUM eviction. The host un-permutes the results.

All layout work (permute, transpose, f32->bf16 cast) happens on the
host so the device kernel is a pure weights-stationary matmul:
  inputs per core:  xT [IN, C] bf16 (tokens transposed), w [IN, OUT]
                    bf16, b [128, OUT/128] f32
  output per core:  outT [OUT, C] bf16  (= (x @ W + b)^T)
"""

import numpy as np
import ml_dtypes

import concourse.bass as bass
import concourse.mybir as mybir
import concourse.tile as tile
from concourse import bacc
from concourse.bass import ts
from concourse.bass_utils import run_bass_kernel_spmd

N_CORES = 8
T = 8           # experts
IN = 1024
OUT = 1024
P = 128
KC = IN // P    # contraction chunks
MC = OUT // P   # output-row chunks
C_DEFAULT = 2176  # token capacity per core (actual max count for the
                  # canonical input is 2088); multiple of 128

_BF16 = ml_dtypes.bfloat16

_nc_cache: dict[int, object] = {}


def _token_chunks(C):
    chunks = []
    off = 0
    while off < C:
        w = min(512, C - off)
        chunks.append((off, w))
        off += w
    return chunks


def _build(C):
    """Build + compile the per-core GEMM program (same on all cores)."""
    nc = bacc.Bacc(
        "TRN2", target_bir_lowering=False, debug=False, num_devices=N_CORES
    )
    xT = nc.dram_tensor("xt", [IN, C], mybir.dt.bfloat16, kind="ExternalInput").ap()
    w = nc.dram_tensor("w", [IN, OUT], mybir.dt.bfloat16, kind="ExternalInput").ap()
    bb = nc.dram_tensor("b", [P, MC], mybir.dt.float32, kind="ExternalInput").ap()
    outT = nc.dram_tensor(
        "outt", [OUT, C], mybir.dt.bfloat16, kind="ExternalOutput"
    ).ap()

    chunks = _token_chunks(C)

    with tile.TileContext(nc) as tc:
        with (
            tc.tile_pool(name="wpool", bufs=1) as wpool,
            tc.tile_pool(name="xpool", bufs=1) as xpool,
            tc.tile_pool(name="bpool", bufs=1) as bpool,
            tc.tile_pool(name="opool", bufs=6) as opool,
            tc.tile_pool(name="psum", bufs=8, space="PSUM") as pspool,
        ):
            b_sb = bpool.tile([P, MC], mybir.dt.float32)
            nc.sync.dma_start(b_sb[:], bb[:])
            # whole weight matrix stays in SBUF: [p, kc, OUT]; per-chunk
            # DMAs so the first matmuls don't wait on the full 2 MiB load
            w_sb = wpool.tile([P, KC, OUT], mybir.dt.bfloat16)
            w_re = w.rearrange("(kc p) o -> p kc o", p=P)
            for k in range(KC):
                nc.sync.dma_start(w_sb[:, k, :], w_re[:, k, :])
            # activations, one tile per contraction chunk for fine DMA deps
            x_sb = []
            for k in range(KC):
                xk = xpool.tile([P, C], mybir.dt.bfloat16, tag=f"x{k}")
                nc.sync.dma_start(xk[:], xT[ts(k, P), :])
                x_sb.append(xk)

            for m in range(MC):
                ptiles = [
                    pspool.tile(
                        [P, 512], mybir.dt.float32, tag="ps", name=f"ps_{m}_{ci}"
                    )
                    for ci in range(len(chunks))
                ]
                for k in range(KC):
                    lhsT = w_sb[:, k, ts(m, P)]
                    for ci, (off, wd) in enumerate(chunks):
                        nc.tensor.matmul(
                            ptiles[ci][:, :wd],
                            lhsT,
                            x_sb[k][:, off : off + wd],
                            start=(k == 0),
                            stop=(k == KC - 1),
                        )
                for ci, (off, wd) in enumerate(chunks):
                    ot = opool.tile([P, 512], mybir.dt.bfloat16, tag="ot")
                    # out = psum + bias[m] (per-partition), cast to bf16
                    nc.scalar.activation(
                        ot[:, :wd],
                        ptiles[ci][:, :wd],
                        mybir.ActivationFunctionType.Identity,
                        bias=b_sb[:, m : m + 1],
                    )
                    nc.sync.dma_start(outT[ts(m, P), off : off + wd], ot[:, :wd])

    nc.compile()
    return nc


def _get_nc(C):
    if C not in _nc_cache:
        _nc_cache[C] = _build(C)
    return _nc_cache[C]


def _route(x, types):
    """Group token indices by expert type."""
    x = np.asarray(x)
    types = np.asarray(types)
    B, S, _ = x.shape
    x_flat = np.ascontiguousarray(x.reshape(B * S, IN))
    t_flat = types.reshape(B * S).astype(np.int64)
    order = np.argsort(t_flat, kind="stable")
    counts = np.bincount(t_flat, minlength=T)
    idx_lists = []
    off = 0
    for t in range(T):
        idx_lists.append(order[off : off + counts[t]])
        off += counts[t]
    return x_flat, idx_lists, counts


def _make_in_maps(x_flat, idx_lists, W, b, C):
    W = np.asarray(W)
    b = np.asarray(b)
    in_maps = []
    for t in range(T):
        idx = idx_lists[t]
        n = len(idx)
        xTt = np.zeros((IN, C), dtype=_BF16)
        if n:
            xTt[:, :n] = x_flat[idx].astype(_BF16).T
        in_maps.append(
            {
                "xt": xTt,
                "w": W[t].astype(_BF16),
                "b": np.ascontiguousarray(
                    b[t].astype(np.float32).reshape(MC, P).T
                ),
            }
        )
    return in_maps


def kernel(x, types, W, b):
    x = np.asarray(x)
    B, S, _ = x.shape
    x_flat, idx_lists, counts = _route(x, types)
    C = max(C_DEFAULT, (int(counts.max()) + P - 1) // P * P)
    nc = _get_nc(C)
    in_maps = _make_in_maps(x_flat, idx_lists, W, b, C)
    res = run_bass_kernel_spmd(nc, in_maps, list(range(N_CORES)), trace=False)
    out_flat = np.empty((B * S, OUT), dtype=np.float32)
    for t in range(T):
        idx = idx_lists[t]
        if len(idx):
            out_flat[idx] = res.results[t]["outt"][:, : len(idx)].T.astype(
                np.float32
            )
    return out_flat.reshape(B, S, OUT)
